# revision 1
# baseline (speedup 1.0000x reference)
"""Trainium2 Bass kernel for nn_Loss_Q_62259845922881 (Q-index loss).

Sharding: band b -> core b (8 bands, 8 cores); each core processes the
4 batch images of its band. Final mean is reduced on host from per-core
per-partition partial sums (8 x [128] floats).

Pipeline per core (per image):
  1. depthwise 41x41 conv via banded matmuls on TensorE (float32r):
     output-row tiles of 88 rows, K=128 input chunks at 88-row stride
  2. fields o^2, l^2, o*l on ACT/DVE
  3. box pass 1 (sum over rows y): field-stationary matmul vs ones band
     -> output arrives transposed [x, y']; N windowed to the band support
  4. box pass 2 (sum over x): band-stationary matmul, shared ones band
  5. quality map on DVE/ACT/GpSimd, fused accum -> per-partition acc
"""

import numpy as np

NB = 8          # bands = cores
B = 4           # batch
MTF = 41        # conv kernel size
BS = 32         # box size
NBOX = float(BS * BS)   # 1024.0
HI, WI = 552, 552       # input spatial
CH = 88         # conv output-row tile stride
NCH = 6         # conv tiles / field row chunks (5x88 + 72 = 512)
HP = CH * 5 + 128       # 568: padded input rows
LP = CH * NCH           # 528: padded label rows
HO, WO = 512, 512       # conv output
QD = 481        # box output = 512 - 32 + 1
QDP = 482       # QD padded even (float32r moving dim must be even)
MT = 97         # box pass-2 output tile rows (last tile 93)

# pass-2 tiles: (m, xs, K): out x' in [xs, xs+m), in x rows [xs, xs+K)
# pass-1 emits x-tiles at exactly these offsets so pass-2 reads are base-0
P2_TILES = []
for tau in range(5):
    m = MT if tau < 4 else QD - 4 * MT
    xs = MT * tau if tau < 4 else HO - (QD - 4 * MT) - (BS - 1)  # 388
    P2_TILES.append((m, xs, m + BS - 1))

# pass-1 N windows per row chunk t: band support of y in [88t, 88t+K1)
# is y' in [88t-31, 88t+K1). float32r matmuls run at 1/4 rate below a
# 256-wide moving dim, so windows are padded to exactly 256 columns.
P1_WIN = []
for t in range(NCH):
    K1 = CH if t < NCH - 1 else HO - CH * (NCH - 1)   # 88 or 72
    w0 = max(0, CH * t - (BS - 1))
    w0 -= w0 % 2
    w0 = min(w0, QDP - 256)
    P1_WIN.append((K1, w0, w0 + 256))


def _build_w1(mtf_band: np.ndarray) -> np.ndarray:
    """Conv band lhsT, layout [r, kx, m]: w1 = mtf[r-m, kx]."""
    w1 = np.zeros((128, MTF, CH), dtype=np.float32)
    for r in range(128):
        for m in range(CH):
            ky = r - m
            if 0 <= ky < MTF:
                w1[r, :, m] = mtf_band[ky, :]
    return w1


def _build_bv() -> np.ndarray:
    """Pass-1 moving ones band [88, 6, QDP]:
    bv[p, t, y'] = 1 iff 0 <= (88t+p) - y' <= 31 (pad col y'=481 zero)."""
    bv = np.zeros((CH, NCH, QDP), dtype=np.float32)
    for t in range(NCH):
        for p in range(CH):
            y = CH * t + p
            if y >= HO:
                continue
            lo = max(0, y - (BS - 1))
            hi = min(QD, y + 1)
            bv[p, t, lo:hi] = 1.0
    return bv


def _build_gp() -> np.ndarray:
    """Pass-2 stationary ones band [128, MT]: gp[r, m] = 1 iff 0 <= r-m <= 31."""
    gp = np.zeros((128, MT), dtype=np.float32)
    for r in range(128):
        for m in range(MT):
            if 0 <= r - m <= BS - 1:
                gp[r, m] = 1.0
    return gp


def build_nc():
    import concourse.bass as bass
    import concourse.tile as tile
    import concourse.mybir as mybir
    from concourse import bacc

    F32 = mybir.dt.float32
    F32R = mybir.dt.float32r
    ALU = mybir.AluOpType

    nc = bacc.Bacc("TRN2", target_bir_lowering=False, debug=False,
                   num_devices=NB)

    x_d = nc.declare_dram_parameter("x", [B, HP, WI], F32R, isOutput=False)
    l_d = nc.declare_dram_parameter("lab", [B, LP, WO], F32R, isOutput=False)
    w1_d = nc.declare_dram_parameter("w1", [128, MTF, CH], F32R, isOutput=False)
    bv_d = nc.declare_dram_parameter("bv", [CH, NCH, QDP], F32R, isOutput=False)
    gp_d = nc.declare_dram_parameter("gp", [128, MT], F32R, isOutput=False)
    acc_d = nc.declare_dram_parameter("acc", [128, 1], F32, isOutput=True)

    with tile.TileContext(nc) as tc:
        with (
            tc.tile_pool(name="wpool", bufs=1) as wpool,
            tc.tile_pool(name="inp", bufs=2) as inp_pool,
            tc.tile_pool(name="lbp", bufs=1) as lb_pool,
            tc.tile_pool(name="fld", bufs=1) as fld_pool,
            tc.tile_pool(name="in2", bufs=1) as in2_pool,
            tc.tile_pool(name="qt", bufs=1) as qt_pool,
            tc.tile_pool(name="accp", bufs=1) as acc_pool,
            tc.tile_pool(name="psc", bufs=3, space=bass.MemorySpace.PSUM) as ps_conv,
            tc.tile_pool(name="ps1", bufs=2, space=bass.MemorySpace.PSUM) as ps_box1,
            tc.tile_pool(name="ps2", bufs=3, space=bass.MemorySpace.PSUM) as ps_box2,
        ):
            # constants (gp + bv first: the PE warmup depends on them)
            gp_sb = wpool.tile([128, MT], F32R, tag="gp")
            nc.sync.dma_start(gp_sb[:], gp_d[:])
            bv_sb = wpool.tile([CH, NCH, QDP], F32R, tag="bv")
            nc.sync.dma_start(bv_sb[:], bv_d[:])
            w1_sb = wpool.tile([128, MTF, CH], F32R, tag="w1")
            nc.sync.dma_start(w1_sb[:], w1_d[:])

            acc_sb = acc_pool.tile([128, 1], F32, tag="acc")
            nc.vector.memset(acc_sb[:], 0.0)

            # PE warmup: keep TensorE busy during the first input DMA so the
            # HAM clock gate is released before the real convolution starts.
            # 12 matmuls ~ matches the image-0 DMA; more delays the conv.
            warm = ps_conv.tile([128, WO], F32, tag="psc", name="warm")
            for _ in range(12):
                nc.tensor.matmul(
                    warm[0:MT, 0:QDP],
                    gp_sb[0:CH, :],
                    bv_sb[0:CH, 0, :],
                    start=True,
                    stop=True,
                )

            for b in range(B):
                # ---- load input (88-row-stride overlapping chunks) ----
                in_sb = inp_pool.tile([128, NCH, WI], F32R, tag="in")
                x_src = bass.AP(x_d, b * HP * WI,
                                [[WI, 128], [CH * WI, NCH], [1, WI]])
                nc.sync.dma_start(in_sb[:], x_src)
                l_sb = lb_pool.tile([CH, NCH, WO], F32R, tag="lab")
                nc.sync.dma_start(
                    l_sb[:], l_d[b].rearrange("(t p) x -> p t x", p=CH))

                # ---- conv: tile T -> out rows [88T, 88T+MTc) ----
                o_sb = fld_pool.tile([CH, NCH, WO], F32R, tag="o")
                for T in range(NCH):
                    MTc = CH if T < NCH - 1 else HO - CH * (NCH - 1)  # 88/72
                    pso = ps_conv.tile([128, WO], F32, tag="psc")
                    for kx in range(MTF):
                        nc.tensor.matmul(
                            pso[0:MTc, :],
                            w1_sb[:, kx, 0:MTc],
                            in_sb[:, T, kx:kx + WO],
                            start=(kx == 0),
                            stop=(kx == MTF - 1),
                        )
                    nc.vector.tensor_copy(o_sb[0:MTc, T, :], pso[0:MTc, :])

                # ---- fields ----
                osq_sb = fld_pool.tile([CH, NCH, WO], F32R, tag="osq")
                nc.scalar.square(osq_sb[:], o_sb[:].bitcast(F32))
                lsq_sb = fld_pool.tile([CH, NCH, WO], F32R, tag="lsq")
                nc.scalar.square(lsq_sb[:], l_sb[:].bitcast(F32))
                ol_sb = fld_pool.tile([CH, NCH, WO], F32R, tag="ol")
                nc.vector.tensor_mul(ol_sb[:], o_sb[:].bitcast(F32),
                                     l_sb[:].bitcast(F32))

                # fields order: a=o_sum b=l_sum c=ol_sum d=osq_sum e=lsq_sum
                fields = [o_sb, l_sb, ol_sb, osq_sb, lsq_sb]

                # ---- box pass 1: out1[x, y'] = sum_y F[y, x] * band ----
                in2 = []
                for f, F_sb in enumerate(fields):
                    i2 = in2_pool.tile([128, 5, QDP], F32R, tag=f"i2_{f}")
                    for tau in range(5):
                        m2, xs, K2 = P2_TILES[tau]
                        mw = K2 if tau == 4 else 128  # x-tile width
                        ps1 = ps_box1.tile([128, QDP], F32, tag="ps1")
                        for t in range(NCH):
                            K1, w0, w1 = P1_WIN[t]
                            nc.tensor.matmul(
                                ps1[0:mw, w0:w1],
                                F_sb[0:K1, t, xs:xs + mw],
                                bv_sb[0:K1, t, w0:w1],
                                start=(t == 0),
                                stop=(t == NCH - 1),
                                skip_group_check=True,
                            )
                        if f % 2 == 0:
                            nc.scalar.copy(i2[0:mw, tau, :], ps1[0:mw, :])
                        else:
                            nc.vector.tensor_copy(i2[0:mw, tau, :], ps1[0:mw, :])
                    in2.append(i2)

                # ---- box pass 2 + quality per x'-tile ----
                for tau in range(5):
                    m, xs, K2 = P2_TILES[tau]
                    sb = []
                    for f in range(5):
                        ps2 = ps_box2.tile([128, QDP], F32, tag="ps2")
                        nc.tensor.matmul(
                            ps2[0:m, :],
                            gp_sb[0:K2, 0:m],
                            in2[f][0:K2, tau, :],
                            start=True,
                            stop=True,
                        )
                        s = qt_pool.tile([128, QDP], F32, tag=f"sb_{f}", bufs=2)
                        if f in (1, 4):
                            nc.vector.tensor_copy(s[0:m, :], ps2[0:m, :])
                        else:
                            nc.scalar.copy(s[0:m, :], ps2[0:m, :])
                        sb.append(s)

                    a, bq, cq, d, e = (s[0:m, :] for s in sb)
                    a2 = qt_pool.tile([128, QDP], F32, tag="a2", name="a2")[0:m, :]
                    nc.scalar.square(a2, a)
                    b2 = qt_pool.tile([128, QDP], F32, tag="b2", name="b2")[0:m, :]
                    nc.scalar.square(b2, bq)
                    mulv = qt_pool.tile([128, QDP], F32, tag="mulv", name="mulv", bufs=2)[0:m, :]
                    nc.vector.tensor_mul(mulv, a, bq)
                    sqv = qt_pool.tile([128, QDP], F32, tag="sqv", name="sqv", bufs=2)[0:m, :]
                    nc.gpsimd.tensor_add(sqv, a2, b2)
                    t1 = qt_pool.tile([128, QDP], F32, tag="t1", name="t1", bufs=2)[0:m, :]
                    nc.vector.scalar_tensor_tensor(
                        t1, cq, NBOX, mulv, ALU.mult, ALU.subtract)
                    numv = qt_pool.tile([128, QDP], F32, tag="numv", name="numv", bufs=2)[0:m, :]
                    nc.gpsimd.tensor_mul(numv, t1, mulv)
                    s2 = qt_pool.tile([128, QDP], F32, tag="s2", name="s2")[0:m, :]
                    nc.vector.tensor_add(s2, d, e)
                    dtv = qt_pool.tile([128, QDP], F32, tag="dtv", name="dtv")[0:m, :]
                    nc.vector.scalar_tensor_tensor(
                        dtv, s2, NBOX, sqv, ALU.mult, ALU.subtract)
                    denv = qt_pool.tile([128, QDP], F32, tag="denv", name="denv")[0:m, :]
                    nc.gpsimd.tensor_mul(denv, dtv, sqv)
                    rv = qt_pool.tile([128, QDP], F32, tag="t1", name="rv", bufs=2)[0:m, :]
                    nc.vector.reciprocal_approx_fast(rv[:, 0:QD], denv[:, 0:QD])
                    qs = qt_pool.tile([128, QDP], F32, tag="mulv", name="qs", bufs=2)[0:m, :]
                    qacc = qt_pool.tile([128, 1], F32, tag="qacc", name="qacc")[0:m, :]
                    nc.vector.scalar_tensor_tensor(
                        qs[:, 0:QD], numv[:, 0:QD], 1.0, rv[:, 0:QD],
                        ALU.mult, ALU.mult, accum_out=qacc)
                    nc.gpsimd.tensor_add(acc_sb[0:m, :], acc_sb[0:m, :], qacc)

            nc.sync.dma_start(acc_d[:], acc_sb[:])

    nc.compile()
    return nc


_NC_CACHE = None


def _get_nc():
    global _NC_CACHE
    if _NC_CACHE is None:
        _NC_CACHE = build_nc()
    return _NC_CACHE


def make_in_maps(outputs, labels, mtf_kernel):
    bv = _build_bv()
    gp = _build_gp()
    in_maps = []
    for band in range(NB):
        xb = np.zeros((B, HP, WI), dtype=np.float32)
        xb[:, :HI, :] = outputs[:, band]
        lb = np.zeros((B, LP, WO), dtype=np.float32)
        lb[:, :HO, :] = labels[:, band]
        in_maps.append({
            "x": np.ascontiguousarray(xb),
            "lab": np.ascontiguousarray(lb),
            "w1": _build_w1(np.asarray(mtf_kernel[band, 0], dtype=np.float32)),
            "bv": bv,
            "gp": gp,
        })
    return in_maps


def run(outputs, labels, mtf_kernel, trace=False):
    import time
    from concourse.bass_utils import run_bass_kernel_spmd
    nc = _get_nc()
    in_maps = make_in_maps(outputs, labels, mtf_kernel)
    res = None
    for attempt in range(3):
        try:
            res = run_bass_kernel_spmd(nc, in_maps, list(range(NB)), trace=trace)
            break
        except Exception:
            # a previously wedged device clears on the next attach; retry
            if attempt == 2:
                raise
            time.sleep(5)
    total = np.float64(0.0)
    for r in res.results:
        total += np.asarray(r["acc"], dtype=np.float64).sum()
    mtot = float(B * NB * QD * QD)
    out = np.asarray(1.0 - 4.0 * total / mtot, dtype=np.float32)
    return out, res


def kernel(outputs, labels, mtf_kernel):
    out, _ = run(outputs, labels, mtf_kernel, trace=False)
    return out


def bench(outputs, labels, mtf_kernel, reps=20, pipeline=None):
    """Time repeated on-device executions with inputs resident on device.

    Returns (min_wall_ns, all_times_ns, result). With pipeline=n, issues n
    unblocked calls and reports the marginal per-call time (closer to pure
    device time; the axon dispatch overhead is ~1.1 ms/call).
    """
    import time
    import jax
    from jax.sharding import Mesh, PartitionSpec, NamedSharding
    from jax.experimental.shard_map import shard_map
    import concourse.mybir as mybir
    from concourse import bass2jax
    from concourse.bass2jax import _bass_exec_p, partition_id_tensor

    bass2jax.install_neuronx_cc_hook()
    nc = _get_nc()
    in_maps = make_in_maps(outputs, labels, mtf_kernel)
    n_cores = NB

    partition_name = nc.partition_id_tensor.name if nc.partition_id_tensor else None
    in_names, out_names, out_avals, zero_outs = [], [], [], []
    for alloc in nc.m.functions[0].allocations:
        if not isinstance(alloc, mybir.MemoryLocationSet):
            continue
        name = alloc.memorylocations[0].name
        if alloc.kind == "ExternalInput":
            if name != partition_name:
                in_names.append(name)
        elif alloc.kind == "ExternalOutput":
            out_names.append(name)
            shape = tuple(alloc.tensor_shape)
            dtype = mybir.dt.np(alloc.dtype)
            out_avals.append(jax.core.ShapedArray(shape, dtype))
            zero_outs.append(np.zeros(shape, dtype))
    n_params = len(in_names)
    n_outs = len(out_avals)
    in_names.extend(out_names)
    if partition_name is not None:
        in_names.append(partition_name)

    donate = tuple(range(n_params, n_params + n_outs))

    def _body(*args):
        operands = list(args)
        if partition_name is not None:
            operands.append(partition_id_tensor())
        outs = _bass_exec_p.bind(
            *operands,
            out_avals=tuple(out_avals),
            in_names=tuple(in_names),
            out_names=tuple(out_names),
            lowering_input_output_aliases=(),
            sim_require_finite=True,
            sim_require_nnan=True,
            nc=nc,
        )
        return tuple(outs)

    devices = jax.devices()[:n_cores]
    mesh = Mesh(np.asarray(devices), ("core",))
    in_specs = (PartitionSpec("core"),) * (n_params + n_outs)
    out_specs = (PartitionSpec("core"),) * len(out_names)
    sharded = jax.jit(
        shard_map(_body, mesh=mesh, in_specs=in_specs, out_specs=out_specs,
                  check_rep=False),
        donate_argnums=donate, keep_unused=True,
    )
    per_core = [[np.asarray(m[name]) for name in in_names[:n_params]]
                for m in in_maps]
    sh = NamedSharding(mesh, PartitionSpec("core"))
    concat_in = [
        jax.device_put(
            np.concatenate([per_core[c][i] for c in range(n_cores)], axis=0), sh)
        for i in range(n_params)
    ]

    def make_zeros():
        return [jax.device_put(
            np.zeros((n_cores * z.shape[0], *z.shape[1:]), z.dtype), sh)
            for z in zero_outs]

    def one_call():
        zeros = make_zeros()
        t0 = time.perf_counter()
        outs = sharded(*concat_in, *zeros)
        jax.block_until_ready(outs)
        return (time.perf_counter() - t0) * 1e9, outs

    one_call()  # compile + warm
    outs = None
    if pipeline:
        def call_async(n):
            zs = [make_zeros() for _ in range(n)]
            t0 = time.perf_counter()
            rets = [sharded(*concat_in, *z) for z in zs]
            jax.block_until_ready(rets)
            return (time.perf_counter() - t0) * 1e9, rets[-1]
        call_async(2)
        t1, _ = call_async(1)
        tn, outs = call_async(pipeline)
        marginal = (tn - t1) / (pipeline - 1)
        times = [t1, tn, marginal]
        tmin = marginal
    else:
        times = []
        for _ in range(reps):
            dt, outs = one_call()
            times.append(dt)
        tmin = min(times)
    arrs = np.asarray(outs[0]).reshape(n_cores, 128, 1)
    total = np.float64(arrs.astype(np.float64).sum())
    mtot = float(B * NB * QD * QD)
    result = np.asarray(1.0 - 4.0 * total / mtot, dtype=np.float32)
    return tmin, times, result

